# revision 1
# baseline (speedup 1.0000x reference)
"""v21: v20 + b0 k-tile-0 loaded via a mini-DMA on the scalar HWDGE ring
(parallel receipt with Q; round 0 otherwise waits the full-K receipt,
second in the serial sync-ring chain). v20: b1 casts spread at queue
positions 14/17/20/23 (one-block placement stalls the in-order DVE
queue on b1 DMA receipts wherever it lands). v18: v14 with a 6-matmul warm-up (3.8us continuous, still > the 3.4us
HAM window; flat loads deliver data by ~10us so the old 10-filler burst
had become the critical path into round 0). Flat loads: partition p holds rows
16p..16p+15 (row-block permutation); softmax is permutation-invariant
over k (V loaded with the same order), and the q permutation is
undone by the output DMA access pattern. (v9: b1 casts deferred (their
position in the in-order DVE queue otherwise stalls round 0 on b1's
input DMA completion).

vs v6: Q/K pre-cast to fp16 on DVE so the PE transposes are single-pass
fp16 (fp32 transposes run as 2 matmul passes); epilogue work is woven
into the next half's rounds as queued jobs instead of filler-padded
blocks; minimal warm-up burst.
"""

import os
from collections import deque

import numpy as np

import concourse.bacc as bacc
import concourse.mybir as mybir
import concourse.tile as tile
from concourse.bass_utils import run_bass_kernel_spmd
from concourse.masks import make_identity

B, N, D = 16, 2048, 64
NCORES = 8
BPC = B // NCORES
TEMP = 8.0

NT = N // 128
F32 = mybir.dt.float32
F16 = mybir.dt.float16

_RESULTS = None


def attention_tile_kernel(tc):
    nc = tc.nc
    q = nc.declare_dram_parameter("q", [BPC, N, D], F32, isOutput=False)
    k = nc.declare_dram_parameter("k", [BPC, N, D], F32, isOutput=False)
    v = nc.declare_dram_parameter("v", [BPC, N, D], F32, isOutput=False)
    out = nc.declare_dram_parameter("out", [BPC, N, D], F32, isOutput=True)

    with (
        tc.tile_pool(name="const", bufs=1) as cpool,
        tc.tile_pool(name="inp", bufs=2) as inp,
        tc.tile_pool(name="qkt", bufs=2) as qkt,
        tc.tile_pool(name="exp", bufs=4) as epool,
        tc.tile_pool(name="outs", bufs=2) as outp,
        tc.tile_pool(name="spsum", bufs=2, space="PSUM") as spool,
        tc.tile_pool(name="opsum", bufs=2, space="PSUM") as opool,
        tc.tile_pool(name="tpsum", bufs=2, space="PSUM") as tpool,
    ):
        ident = cpool.tile([128, 128], F32)
        make_identity(nc, ident)
        ident16 = cpool.tile([128, 128], F16)
        nc.vector.tensor_copy(ident16, ident)
        wsrc = cpool.tile([128, 512], F32)
        nc.vector.memset(wsrc, 0.0)
        warm_w = cpool.tile([64, 128], F16)
        nc.vector.tensor_copy(warm_w, wsrc[0:64, 0:128])
        warm_r = cpool.tile([64, 512], F16)
        nc.vector.tensor_copy(warm_r, wsrc[0:64, :])
        warm_sink = cpool.tile([128, 1], F32)

        def tptile(name):
            # one PSUM bank, viewed as fp16 [128,1024] or f32 [128,512]
            return tpool.tile([128, 1024], F16, tag="tp", name=name)

        def filler(tp):
            nc.tensor.matmul(
                tp.bitcast(F32), warm_w, warm_r, start=True, stop=True
            )

        # ---- input loads, both batches (b1's DVE casts deferred) ----
        vp, qf, kf, q_t, k_t = [], [], [], [], []
        cast_later = []
        for b in range(BPC):
            qn = inp.tile([128, NT, D], F32, tag="qnat", name=f"qn{b}")
            kn = inp.tile([128, NT, D], F32, tag="knat", name=f"kn{b}")
            vpb = inp.tile([128, NT, D + 1], F16, tag="vp", name=f"vp{b}")
            # flat loads: 4KB contiguous per partition (full DMA rate);
            # partition p <- rows 16p..16p+15, i.e. tile r holds rows {16p+r}
            nc.sync.dma_start(out=qn, in_=q[b].rearrange("(p r) d -> p r d", p=128))
            kview = k[b].rearrange("(p r) d -> p r d", p=128)
            if b == 0:
                # round 0 needs only k-tile 0: land it on the scalar HWDGE
                # ring (parallel receipt; no ACTIVATEs queued yet)
                nc.scalar.dma_start(out=kn[:, 0:1, :], in_=kview[:, 0:1, :])
                nc.sync.dma_start(out=kn[:, 1:NT, :], in_=kview[:, 1:NT, :])
            else:
                nc.sync.dma_start(out=kn, in_=kview)
            nc.gpsimd.dma_start(
                out=vpb[:, :, 0:D], in_=v[b].rearrange("(p r) d -> p r d", p=128)
            )
            ones16 = inp.tile([128, NT], F32, tag="ones16", name=f"on{b}")
            nc.vector.memset(ones16, 1.0)
            nc.vector.tensor_copy(vpb[:, :, D], ones16)
            qfb = inp.tile([128, NT, D], F16, tag="qf16", name=f"qf{b}")
            kfb = inp.tile([128, NT, D], F16, tag="kf16", name=f"kf{b}")
            if b == 0:
                nc.vector.tensor_copy(kfb[:, 0:1, :], kn[:, 0:1, :])  # tiny, first
                nc.vector.tensor_copy(qfb, qn)
                nc.vector.tensor_copy(kfb[:, 1:NT, :], kn[:, 1:NT, :])
            else:
                cast_later.append((qfb, qn, kfb, kn))
            vp.append(vpb)
            qf.append(qfb)
            kf.append(kfb)
            q_t.append(qkt.tile([64, N], F16, tag="qt", name=f"qt{b}"))
            k_t.append(qkt.tile([64, N], F16, tag="kt", name=f"kt{b}"))

        def transpose_job(nat, tmat, t):
            tp = tptile(f"tp{nc.next_id()}")
            nc.tensor.transpose(tp[0:D, 0:128], nat[:, t, :], ident16)
            nc.vector.tensor_copy(tmat[:, t * 128 : (t + 1) * 128], tp[0:D, 0:128])

        def cast_one(dst, srcv):
            return lambda: nc.vector.tensor_copy(dst, srcv)

        b1_cast_jobs = []
        for qfb, qn, kfb, kn in cast_later:
            hv = NT // 2
            b1_cast_jobs += [
                cast_one(qfb[:, 0:hv, :], qn[:, 0:hv, :]),
                cast_one(qfb[:, hv:NT, :], qn[:, hv:NT, :]),
                cast_one(kfb[:, 0:hv, :], kn[:, 0:hv, :]),
                cast_one(kfb[:, hv:NT, :], kn[:, hv:NT, :]),
            ]

        jobs = deque()
        for t in range(8):
            jobs.append((qf[0], q_t[0], t))
        for t in range(NT):
            jobs.append((kf[0], k_t[0], t))
        for pos, cj in zip((23, 20, 17, 14), reversed(b1_cast_jobs)):
            jobs.insert(pos, cj)  # spread: receipts landed, waits pre-satisfied
        for t in range(8, NT):
            jobs.append((qf[0], q_t[0], t))
        for t in range(8):
            jobs.append((qf[1], q_t[1], t))
    
        for t in range(NT):
            jobs.append((kf[1], k_t[1], t))
        for t in range(8, NT):
            jobs.append((qf[1], q_t[1], t))

        def run_job(j):
            if callable(j):
                j()
            else:
                transpose_job(*j)

        # warm-up burst + b0's first-needed transposes
        wps = tptile("wps")
        for _ in range(6):
            filler(wps)
        nc.vector.tensor_copy(warm_sink, wps.bitcast(F32)[:, 0:1])
        for _ in range(9):  # b0: q0..7, k0
            run_job(jobs.popleft())

        # ---- rounds ----
        for b in range(BPC):
            for h in range(2):
                qoff = h * 1024
                out_ps = [
                    opool.tile([D + 1, 512], F32, tag="ops", name=f"o{b}{h}{c}")
                    for c in range(2)
                ]
                pending = []

                def emit_out(pend, out_ps=out_ps, b=b):
                    for kb, e in pend:
                        for c in range(2):
                            nc.tensor.matmul(
                                out_ps[c],
                                vp[b][:, kb, :],
                                e[:, c * 512 : (c + 1) * 512],
                                start=(kb == 0),
                                stop=(kb == NT - 1),
                            )

                for kb in range(NT):
                    s = spool.tile([128, 1024], F32, tag="s")
                    lhs = k_t[b][:, kb * 128 : (kb + 1) * 128]
                    for c in range(2):
                        nc.tensor.matmul(
                            s[:, c * 512 : (c + 1) * 512],
                            lhs,
                            q_t[b][:, qoff + c * 512 : qoff + (c + 1) * 512],
                            start=True,
                            stop=True,
                        )
                    for _ in range(2):
                        if jobs:
                            run_job(jobs.popleft())
                    emit_out(pending)
                    pending = []
                    e = epool.tile([128, 1024], F16, tag="e")
                    nc.scalar.activation(
                        out=e,
                        in_=s,
                        func=mybir.ActivationFunctionType.Exp,
                        scale=1.0 / TEMP,
                    )
                    pending.append((kb, e))
                emit_out(pending)

                # epilogue: copy PSUM accumulators out now (frees opool),
                # queue the normalize+transpose work into later rounds
                for c in range(2):
                    qc = h * 2 + c
                    osb = outp.tile([D + 1, 512], F32, tag="osb", name=f"os{b}{qc}")
                    nc.vector.tensor_copy(osb, out_ps[c])
                    fout = outp.tile([128, 4, D], F32, tag="fout", name=f"fo{b}{qc}")

                    def ep_job(j, osb=osb, fout=fout):
                        tp = tptile(f"ep{nc.next_id()}")
                        tpf = tp.bitcast(F32)
                        nc.tensor.transpose(
                            tpf[:, 0 : D + 1],
                            osb[:, j * 128 : (j + 1) * 128],
                            ident[0 : D + 1, 0 : D + 1],
                        )
                        rcp = outp.tile([128, 1], F32, tag="rcp")
                        nc.vector.reciprocal(rcp, tpf[:, D : D + 1])
                        nc.vector.tensor_scalar_mul(
                            out=fout[:, j, :], in0=tpf[:, 0:D], scalar1=rcp
                        )

                    def dma_job(b=b, qc=qc, fout=fout):
                        nc.sync.dma_start(
                            out=out[b].rearrange("(p r) d -> p r d", p=128)[
                                :, 4 * qc : 4 * qc + 4, :
                            ],
                            in_=fout,
                        )

                    for j in range(4):
                        jobs.append((lambda j=j, f=ep_job: f(j)))
                    jobs.append(dma_job)

        # drain any remaining jobs (last half's epilogue, tail transposes)
        while jobs:
            run_job(jobs.popleft())
    return nc


def build_program():
    nc = bacc.Bacc(None)
    with tile.TileContext(nc) as tc:
        attention_tile_kernel(tc)
    nc.finalize()
    return nc


def kernel(queries: np.ndarray, keys: np.ndarray, values: np.ndarray) -> np.ndarray:
    global _RESULTS
    queries = np.ascontiguousarray(queries, dtype=np.float32)
    keys = np.ascontiguousarray(keys, dtype=np.float32)
    values = np.ascontiguousarray(values, dtype=np.float32)
    nc = build_program()
    in_maps = [
        {
            "q": queries[i * BPC : (i + 1) * BPC],
            "k": keys[i * BPC : (i + 1) * BPC],
            "v": values[i * BPC : (i + 1) * BPC],
        }
        for i in range(NCORES)
    ]
    trace = bool(os.environ.get("ATTN_TRACE"))
    if trace:
        _register_ntff_hook()
    _RESULTS = run_bass_kernel_spmd(nc, in_maps, list(range(NCORES)), trace=trace)
    return np.concatenate([r["out"] for r in _RESULTS.results], axis=0)


def _register_ntff_hook():
    """Dev-only: the slim agent container lacks antenv.axon_hooks; provide it
    so run_bass_kernel_spmd(trace=True) can drive NRT profiling via the axon
    .so directly. No-op unless ATTN_TRACE is set."""
    import sys
    import types

    if "antenv.axon_hooks" in sys.modules:
        return
    try:
        from trn_agent_boot.trn_boot import _ntff_profile_via_ctypes

        h = _ntff_profile_via_ctypes("/opt/axon/libaxon_pjrt.so")
    except Exception:
        return
    mod = types.ModuleType("antenv.axon_hooks")
    mod.get_axon_ntff_profile_hook = lambda: h
    sys.modules["antenv.axon_hooks"] = mod



# revision 2
# speedup vs baseline: 1.1667x; 1.1667x over previous
"""v22: flat-round pipeline on top of v21.

- Out-matmuls deferred 2 rounds (3 for kb=0) behind their exp so the PE
  never waits on the Act engine; pending carries across half boundaries.
- Warm-up tiles built by gpsimd memset (no DVE dependency) so fillers
  start right after the preamble; V's ones-column is a gpsimd memset.
- b0 casts split (k0-tile, q lo, q hi, k rest) so round-0 transposes
  start earlier; b1 casts run on the Pool engine as queued jobs (DVE
  freed for transpose copies).
- Epilogue in fp16: osb copied f32->f16, transposes single-pass fp16
  (65 cols vs 130 fp32 col-passes), recip in f32 from the f16 row,
  jobs bundled 2-per-slot; final half's muls go on the idle Act engine.
- 3 jobs popped per round late in the schedule so the job queue is dry
  when the last half's epilogue begins.
"""

import os
from collections import deque

import numpy as np

import concourse.bacc as bacc
import concourse.mybir as mybir
import concourse.tile as tile
from concourse.bass_utils import run_bass_kernel_spmd
from concourse.masks import make_identity

B, N, D = 16, 2048, 64
NCORES = 8
BPC = B // NCORES
TEMP = 8.0

NT = N // 128
F32 = mybir.dt.float32
F16 = mybir.dt.float16

_RESULTS = None


def attention_tile_kernel(tc):
    nc = tc.nc
    q = nc.declare_dram_parameter("q", [BPC, N, D], F32, isOutput=False)
    k = nc.declare_dram_parameter("k", [BPC, N, D], F32, isOutput=False)
    v = nc.declare_dram_parameter("v", [BPC, N, D], F32, isOutput=False)
    out = nc.declare_dram_parameter("out", [BPC, N, D], F32, isOutput=True)

    with (
        tc.tile_pool(name="const", bufs=1) as cpool,
        tc.tile_pool(name="inp", bufs=2) as inp,
        tc.tile_pool(name="qkt", bufs=2) as qkt,
        tc.tile_pool(name="exp", bufs=4) as epool,
        tc.tile_pool(name="outs", bufs=2) as outp,
        tc.tile_pool(name="spsum", bufs=2, space="PSUM") as spool,
        tc.tile_pool(name="opsum", bufs=2, space="PSUM") as opool,
        tc.tile_pool(name="tpsum", bufs=2, space="PSUM") as tpool,
    ):
        ident = cpool.tile([128, 128], F32)
        make_identity(nc, ident)
        ident16 = cpool.tile([128, 128], F16)
        nc.vector.tensor_copy(ident16, ident)
        # warm tiles via Pool memsets: fillers have no DVE dependency
        warm_w = cpool.tile([64, 128], F16)
        nc.gpsimd.memset(warm_w, 0.0)
        warm_r = cpool.tile([64, 512], F16)
        nc.gpsimd.memset(warm_r, 0.0)
        warm_sink = cpool.tile([128, 1], F32)

        def tptile(name):
            # one PSUM bank, viewed as fp16 [128,1024] or f32 [128,512]
            return tpool.tile([128, 1024], F16, tag="tp", name=name)

        def filler(tp):
            nc.tensor.matmul(
                tp.bitcast(F32), warm_w, warm_r, start=True, stop=True
            )

        # ---- input loads, both batches (b1's casts deferred to Pool) ----
        vp, qf, kf, q_t, k_t = [], [], [], [], []
        cast_later = []
        for b in range(BPC):
            qn = inp.tile([128, NT, D], F32, tag="qnat", name=f"qn{b}")
            kn = inp.tile([128, NT, D], F32, tag="knat", name=f"kn{b}")
            vpb = inp.tile([128, NT, D + 1], F16, tag="vp", name=f"vp{b}")
            # flat loads: 4KB contiguous per partition (full DMA rate);
            # partition p <- rows 16p..16p+15, i.e. tile r holds rows {16p+r}
            nc.sync.dma_start(out=qn, in_=q[b].rearrange("(p r) d -> p r d", p=128))
            kview = k[b].rearrange("(p r) d -> p r d", p=128)
            if b == 0:
                # round 0 needs only k-tile 0: land it on the scalar HWDGE
                # ring (parallel receipt; no ACTIVATEs queued yet)
                nc.scalar.dma_start(out=kn[:, 0:1, :], in_=kview[:, 0:1, :])
                nc.sync.dma_start(out=kn[:, 1:NT, :], in_=kview[:, 1:NT, :])
            else:
                nc.sync.dma_start(out=kn, in_=kview)
            nc.gpsimd.dma_start(
                out=vpb[:, :, 0:D], in_=v[b].rearrange("(p r) d -> p r d", p=128)
            )
            nc.gpsimd.memset(vpb[:, :, D], 1.0)
            qfb = inp.tile([128, NT, D], F16, tag="qf16", name=f"qf{b}")
            kfb = inp.tile([128, NT, D], F16, tag="kf16", name=f"kf{b}")
            if b == 0:
                nc.vector.tensor_copy(kfb[:, 0:1, :], kn[:, 0:1, :])  # tiny, first
                nc.vector.tensor_copy(qfb[:, 0:8, :], qn[:, 0:8, :])
                nc.vector.tensor_copy(qfb[:, 8:NT, :], qn[:, 8:NT, :])
                nc.vector.tensor_copy(kfb[:, 1:NT, :], kn[:, 1:NT, :])
            else:
                cast_later.append((qfb, qn, kfb, kn))
            vp.append(vpb)
            qf.append(qfb)
            kf.append(kfb)
            q_t.append(qkt.tile([64, N], F16, tag="qt", name=f"qt{b}"))
            k_t.append(qkt.tile([64, N], F16, tag="kt", name=f"kt{b}"))

        def transpose_job(nat, tmat, t):
            tp = tptile(f"tp{nc.next_id()}")
            nc.tensor.transpose(tp[0:D, 0:128], nat[:, t, :], ident16)
            nc.vector.tensor_copy(tmat[:, t * 128 : (t + 1) * 128], tp[0:D, 0:128])

        def cast_one(dst, srcv):
            # b1 casts on the Pool engine: DVE stays free for tp copies
            return lambda: nc.gpsimd.tensor_copy(dst, srcv)

        b1_cast_jobs = []
        for qfb, qn, kfb, kn in cast_later:
            hv = NT // 2
            b1_cast_jobs += [
                cast_one(qfb[:, 0:hv, :], qn[:, 0:hv, :]),
                cast_one(qfb[:, hv:NT, :], qn[:, hv:NT, :]),
                cast_one(kfb[:, 0:hv, :], kn[:, 0:hv, :]),
                cast_one(kfb[:, hv:NT, :], kn[:, hv:NT, :]),
            ]

        jobs = deque()
        for t in range(8):
            jobs.append((qf[0], q_t[0], t))
        for t in range(NT):
            jobs.append((kf[0], k_t[0], t))
        for pos, cj in zip((23, 20, 17, 14), reversed(b1_cast_jobs)):
            jobs.insert(pos, cj)  # spread: receipts landed, waits pre-satisfied
        for t in range(8, NT):
            jobs.append((qf[0], q_t[0], t))
        for t in range(8):
            jobs.append((qf[1], q_t[1], t))
        for t in range(NT):
            jobs.append((kf[1], k_t[1], t))
        for t in range(8, NT):
            jobs.append((qf[1], q_t[1], t))

        def run_job(j):
            if callable(j):
                j()
            else:
                transpose_job(*j)

        # warm-up burst + b0's first-needed transposes
        wps = tptile("wps")
        for _ in range(6):
            filler(wps)
        nc.vector.tensor_copy(warm_sink, wps.bitcast(F32)[:, 0:1])
        for _ in range(9):  # b0: q0..7, k0
            run_job(jobs.popleft())

        # ---- epilogue job factory (fp16 path, bundled jobs) ----
        def queue_epilogue(b, h, out_ps, last):
            for c in range(2):
                qc = h * 2 + c
                osb = outp.tile([D + 1, 512], F16, tag="osb", name=f"os{b}{qc}")
                fout = outp.tile([128, 4, D], F32, tag="fout", name=f"fo{b}{qc}")
                tps = {}

                def osb_job(osb=osb, out_ps=out_ps, c=c):
                    nc.vector.tensor_copy(osb, out_ps[c])

                def tr_job(jj, osb=osb, tps=tps):
                    for j in (jj, jj + 1):
                        tp = tptile(f"ep{nc.next_id()}")
                        tps[j] = tp
                        nc.tensor.transpose(
                            tp[:, 0 : D + 1],
                            osb[:, j * 128 : (j + 1) * 128],
                            ident16[0 : D + 1, 0 : D + 1],
                        )

                def norm_job(jj, tps=tps, fout=fout, last=last):
                    for j in (jj, jj + 1):
                        tp = tps[j]
                        rcp = outp.tile(
                            [128, 1], F32, tag="rcp", name=f"rc{nc.next_id()}"
                        )
                        nc.vector.reciprocal(rcp, tp[:, D : D + 1])
                        if last:
                            nc.scalar.activation(
                                out=fout[:, j, :],
                                in_=tp[:, 0:D],
                                func=mybir.ActivationFunctionType.Copy,
                                scale=rcp,
                            )
                        else:
                            nc.vector.tensor_scalar_mul(
                                out=fout[:, j, :], in0=tp[:, 0:D], scalar1=rcp
                            )

                def dma_job(b=b, qc=qc, fout=fout):
                    nc.sync.dma_start(
                        out=out[b].rearrange("(p r) d -> p r d", p=128)[
                            :, 4 * qc : 4 * qc + 4, :
                        ],
                        in_=fout,
                    )

                jobs.append(osb_job)
                jobs.append(lambda f=tr_job: f(0))
                jobs.append(lambda f=norm_job: f(0))
                jobs.append(lambda f=tr_job: f(2))
                jobs.append(lambda f=norm_job: f(2))
                jobs.append(dma_job)

        # ---- flat rounds with deferred out-matmuls ----
        rounds = [(b, h, kb) for b in range(BPC) for h in range(2) for kb in range(NT)]
        pending = deque()
        out_ps_cur = None

        def emit_pending(p):
            b, h, kb, e, ops = p["b"], p["h"], p["kb"], p["e"], p["ops"]
            for c in range(2):
                nc.tensor.matmul(
                    ops[c],
                    vp[b][:, kb, :],
                    e[:, c * 512 : (c + 1) * 512],
                    start=(kb == 0),
                    stop=(kb == NT - 1),
                )
            if kb == NT - 1:
                queue_epilogue(b, h, ops, last=(b == BPC - 1 and h == 1))

        for r, (b, h, kb) in enumerate(rounds):
            if kb == 0:
                out_ps_cur = [
                    opool.tile([D + 1, 512], F32, tag="ops", name=f"o{b}{h}{c}")
                    for c in range(2)
                ]
            s = spool.tile([128, 1024], F32, tag="s")
            lhs = k_t[b][:, kb * 128 : (kb + 1) * 128]
            for c in range(2):
                nc.tensor.matmul(
                    s[:, c * 512 : (c + 1) * 512],
                    lhs,
                    q_t[b][:, h * 1024 + c * 512 : h * 1024 + (c + 1) * 512],
                    start=True,
                    stop=True,
                )
            for _ in range(3 if r >= 40 else 2):
                if jobs:
                    run_job(jobs.popleft())
            while pending and pending[0]["due"] <= r:
                emit_pending(pending.popleft())
            e = epool.tile([128, 1024], F16, tag="e")
            nc.scalar.activation(
                out=e,
                in_=s,
                func=mybir.ActivationFunctionType.Exp,
                scale=1.0 / TEMP,
            )
            pending.append(
                {
                    "due": r + (3 if kb == 0 else 2),
                    "b": b,
                    "h": h,
                    "kb": kb,
                    "e": e,
                    "ops": out_ps_cur,
                }
            )

        while pending:
            emit_pending(pending.popleft())
        while jobs:
            run_job(jobs.popleft())
    return nc


def build_program():
    nc = bacc.Bacc(None)
    with tile.TileContext(nc) as tc:
        attention_tile_kernel(tc)
    nc.finalize()
    return nc


def kernel(queries: np.ndarray, keys: np.ndarray, values: np.ndarray) -> np.ndarray:
    global _RESULTS
    queries = np.ascontiguousarray(queries, dtype=np.float32)
    keys = np.ascontiguousarray(keys, dtype=np.float32)
    values = np.ascontiguousarray(values, dtype=np.float32)
    nc = build_program()
    in_maps = [
        {
            "q": queries[i * BPC : (i + 1) * BPC],
            "k": keys[i * BPC : (i + 1) * BPC],
            "v": values[i * BPC : (i + 1) * BPC],
        }
        for i in range(NCORES)
    ]
    trace = bool(os.environ.get("ATTN_TRACE"))
    if trace:
        _register_ntff_hook()
    _RESULTS = run_bass_kernel_spmd(nc, in_maps, list(range(NCORES)), trace=trace)
    return np.concatenate([r["out"] for r in _RESULTS.results], axis=0)


def _register_ntff_hook():
    """Dev-only: the slim agent container lacks antenv.axon_hooks; provide it
    so run_bass_kernel_spmd(trace=True) can drive NRT profiling via the axon
    .so directly. No-op unless ATTN_TRACE is set."""
    import sys
    import types

    if "antenv.axon_hooks" in sys.modules:
        return
    try:
        from trn_agent_boot.trn_boot import _ntff_profile_via_ctypes

        h = _ntff_profile_via_ctypes("/opt/axon/libaxon_pjrt.so")
    except Exception:
        return
    mod = types.ModuleType("antenv.axon_hooks")
    mod.get_axon_ntff_profile_hook = lambda: h
    sys.modules["antenv.axon_hooks"] = mod


# revision 3
# speedup vs baseline: 1.1969x; 1.0258x over previous
"""v29: v22 + elastic out-matmul backlog.

The clock governor holds the PE at 1.2GHz for the first ~28-44us. In
v22 every round carries QK + out matmuls (2048 cols), so pre-latch
rounds run ~1.95us while the Act engine (the real bottleneck, 1.11us
of exp per round) idles ~0.8us/round. v29 emits NO out-matmuls for the
first 12 rounds -- pre-latch rounds are QK + one transpose job
(~1.17us, Act-paced) -- and drains the deferred out-matmul backlog
after the clock latches, sized to the per-round PE slack (an extra
pair on job-free rounds). Deferred columns execute at 2.4GHz instead
of 1.2GHz and the Act engine stays saturated from first to last exp,
which also makes the total largely insensitive to the governor's
latch-time variance. e-tiles are held in a 20-deep pool to cover the
backlog; epilogue osb copies run inline at accumulator completion so
the next half's PSUM banks free in time.

From v22: warm-up via gpsimd memsets; b1 casts on Pool; fp16 epilogue,
bundled jobs, last half's muls on the Act engine.
"""

import os
from collections import deque

import numpy as np

import concourse.bacc as bacc
import concourse.mybir as mybir
import concourse.tile as tile
from concourse.bass_utils import run_bass_kernel_spmd
from concourse.masks import make_identity

B, N, D = 16, 2048, 64
NCORES = 8
BPC = B // NCORES
TEMP = 8.0

NT = N // 128
F32 = mybir.dt.float32
F16 = mybir.dt.float16

_RESULTS = None


def attention_tile_kernel(tc):
    nc = tc.nc
    q = nc.declare_dram_parameter("q", [BPC, N, D], F32, isOutput=False)
    k = nc.declare_dram_parameter("k", [BPC, N, D], F32, isOutput=False)
    v = nc.declare_dram_parameter("v", [BPC, N, D], F32, isOutput=False)
    out = nc.declare_dram_parameter("out", [BPC, N, D], F32, isOutput=True)

    with (
        tc.tile_pool(name="const", bufs=1) as cpool,
        tc.tile_pool(name="inp", bufs=2) as inp,
        tc.tile_pool(name="qkt", bufs=2) as qkt,
        tc.tile_pool(name="exp", bufs=20) as epool,
        tc.tile_pool(name="outs", bufs=2) as outp,
        tc.tile_pool(name="spsum", bufs=2, space="PSUM") as spool,
        tc.tile_pool(name="opsum", bufs=2, space="PSUM") as opool,
        tc.tile_pool(name="tpsum", bufs=2, space="PSUM") as tpool,
    ):
        ident = cpool.tile([128, 128], F32)
        make_identity(nc, ident)
        ident16 = cpool.tile([128, 128], F16)
        nc.vector.tensor_copy(ident16, ident)
        # warm tiles via Pool memsets: fillers have no DVE dependency
        warm_w = cpool.tile([64, 128], F16)
        nc.gpsimd.memset(warm_w, 0.0)
        warm_r = cpool.tile([64, 512], F16)
        nc.gpsimd.memset(warm_r, 0.0)
        warm_sink = cpool.tile([128, 1], F32)

        def tptile(name):
            # one PSUM bank, viewed as fp16 [128,1024] or f32 [128,512]
            return tpool.tile([128, 1024], F16, tag="tp", name=name)

        def filler(tp):
            nc.tensor.matmul(
                tp.bitcast(F32), warm_w, warm_r, start=True, stop=True
            )

        # ---- input loads, both batches (b1's casts deferred to Pool) ----
        vp, qf, kf, q_t, k_t = [], [], [], [], []
        cast_later = []
        for b in range(BPC):
            qn = inp.tile([128, NT, D], F32, tag="qnat", name=f"qn{b}")
            kn = inp.tile([128, NT, D], F32, tag="knat", name=f"kn{b}")
            vpb = inp.tile([128, NT, D + 1], F16, tag="vp", name=f"vp{b}")
            # flat loads: 4KB contiguous per partition (full DMA rate);
            # partition p <- rows 16p..16p+15, i.e. tile r holds rows {16p+r}
            nc.sync.dma_start(out=qn, in_=q[b].rearrange("(p r) d -> p r d", p=128))
            kview = k[b].rearrange("(p r) d -> p r d", p=128)
            if b == 0:
                # round 0 needs only k-tile 0: land it on the scalar HWDGE
                # ring (parallel receipt; no ACTIVATEs queued yet)
                nc.scalar.dma_start(out=kn[:, 0:1, :], in_=kview[:, 0:1, :])
                nc.sync.dma_start(out=kn[:, 1:NT, :], in_=kview[:, 1:NT, :])
            else:
                nc.sync.dma_start(out=kn, in_=kview)
            nc.gpsimd.dma_start(
                out=vpb[:, :, 0:D], in_=v[b].rearrange("(p r) d -> p r d", p=128)
            )
            nc.gpsimd.memset(vpb[:, :, D], 1.0)
            qfb = inp.tile([128, NT, D], F16, tag="qf16", name=f"qf{b}")
            kfb = inp.tile([128, NT, D], F16, tag="kf16", name=f"kf{b}")
            if b == 0:
                nc.vector.tensor_copy(kfb[:, 0:1, :], kn[:, 0:1, :])  # tiny, first
                nc.vector.tensor_copy(qfb[:, 0:8, :], qn[:, 0:8, :])
                nc.vector.tensor_copy(qfb[:, 8:NT, :], qn[:, 8:NT, :])
                nc.vector.tensor_copy(kfb[:, 1:NT, :], kn[:, 1:NT, :])
            else:
                cast_later.append((qfb, qn, kfb, kn))
            vp.append(vpb)
            qf.append(qfb)
            kf.append(kfb)
            q_t.append(qkt.tile([64, N], F16, tag="qt", name=f"qt{b}"))
            k_t.append(qkt.tile([64, N], F16, tag="kt", name=f"kt{b}"))

        def transpose_job(nat, tmat, t):
            tp = tptile(f"tp{nc.next_id()}")
            nc.tensor.transpose(tp[0:D, 0:128], nat[:, t, :], ident16)
            nc.vector.tensor_copy(tmat[:, t * 128 : (t + 1) * 128], tp[0:D, 0:128])

        def cast_one(dst, srcv):
            # b1 casts on the Pool engine: DVE stays free for tp copies
            return lambda: nc.gpsimd.tensor_copy(dst, srcv)

        b1_cast_jobs = []
        for qfb, qn, kfb, kn in cast_later:
            hv = NT // 2
            b1_cast_jobs += [
                cast_one(qfb[:, 0:hv, :], qn[:, 0:hv, :]),
                cast_one(qfb[:, hv:NT, :], qn[:, hv:NT, :]),
                cast_one(kfb[:, 0:hv, :], kn[:, 0:hv, :]),
                cast_one(kfb[:, hv:NT, :], kn[:, hv:NT, :]),
            ]

        upfront = [(qf[0], q_t[0], t) for t in range(8)] + [(kf[0], k_t[0], 0)]
        jobs = deque()
        for t in range(1, NT):
            jobs.append((kf[0], k_t[0], t))  # k0 tiles feed rounds 1..15
        for t in range(8, NT):
            jobs.append((qf[0], q_t[0], t))  # b0-h1 q, needed from round 16
        for cj in b1_cast_jobs:
            jobs.append(cj)
        for t in range(NT):
            jobs.append((qf[1], q_t[1], t))
        for t in range(NT):
            jobs.append((kf[1], k_t[1], t))

        def run_job(j):
            if callable(j):
                j()
            else:
                transpose_job(*j)

        # warm-up burst + b0's first-needed transposes
        wps = tptile("wps")
        for _ in range(6):
            filler(wps)
        nc.vector.tensor_copy(warm_sink, wps.bitcast(F32)[:, 0:1])
        for j in upfront:  # b0: q0..7, k0
            run_job(j)

        # ---- epilogue job factory (fp16 path, bundled jobs) ----
        def queue_epilogue(b, h, out_ps, last):
            for c in range(2):
                qc = h * 2 + c
                osb = outp.tile(
                    [D + 1, 512], F16, tag="osb", name=f"os{b}{qc}", bufs=4
                )
                fout = outp.tile(
                    [128, 4, D], F32, tag="fout", name=f"fo{b}{qc}", bufs=4
                )
                tps = {}

                # inline: frees this half's PSUM banks before the next
                # half's first out-matmul needs them
                nc.vector.tensor_copy(osb, out_ps[c])

                def tr_job(jj, osb=osb, tps=tps):
                    for j in (jj, jj + 1):
                        tp = tptile(f"ep{nc.next_id()}")
                        tps[j] = tp
                        nc.tensor.transpose(
                            tp[:, 0 : D + 1],
                            osb[:, j * 128 : (j + 1) * 128],
                            ident16[0 : D + 1, 0 : D + 1],
                        )

                def norm_job(jj, tps=tps, fout=fout, last=last):
                    for j in (jj, jj + 1):
                        tp = tps[j]
                        rcp = outp.tile(
                            [128, 1], F32, tag="rcp", name=f"rc{nc.next_id()}", bufs=4
                        )
                        nc.vector.reciprocal(rcp, tp[:, D : D + 1])
                        if last:
                            nc.scalar.activation(
                                out=fout[:, j, :],
                                in_=tp[:, 0:D],
                                func=mybir.ActivationFunctionType.Copy,
                                scale=rcp,
                            )
                        else:
                            nc.vector.tensor_scalar_mul(
                                out=fout[:, j, :], in0=tp[:, 0:D], scalar1=rcp
                            )

                def dma_job(b=b, qc=qc, fout=fout):
                    nc.sync.dma_start(
                        out=out[b].rearrange("(p r) d -> p r d", p=128)[
                            :, 4 * qc : 4 * qc + 4, :
                        ],
                        in_=fout,
                    )

                jobs.append(lambda f=tr_job: f(0))
                jobs.append(lambda f=norm_job: f(0))
                jobs.append(lambda f=tr_job: f(2))
                jobs.append(lambda f=norm_job: f(2))
                jobs.append(dma_job)

        # ---- flat rounds with elastic out-matmul backlog ----
        rounds = [(b, h, kb) for b in range(BPC) for h in range(2) for kb in range(NT)]
        pending = deque()
        ops_by_half = {}

        def emit_pending(p):
            b, h, kb, e = p["b"], p["h"], p["kb"], p["e"]
            if kb == 0:
                ops_by_half[(b, h)] = [
                    opool.tile([D + 1, 512], F32, tag="ops", name=f"o{b}{h}{c}")
                    for c in range(2)
                ]
            ops = ops_by_half[(b, h)]
            for c in range(2):
                nc.tensor.matmul(
                    ops[c],
                    vp[b][:, kb, :],
                    e[:, c * 512 : (c + 1) * 512],
                    start=(kb == 0),
                    stop=(kb == NT - 1),
                )
            if kb == NT - 1:
                queue_epilogue(b, h, ops, last=(b == BPC - 1 and h == 1))

        for r, (b, h, kb) in enumerate(rounds):
            s = spool.tile([128, 1024], F32, tag="s")
            lhs = k_t[b][:, kb * 128 : (kb + 1) * 128]
            for c in range(2):
                nc.tensor.matmul(
                    s[:, c * 512 : (c + 1) * 512],
                    lhs,
                    q_t[b][:, h * 1024 + c * 512 : h * 1024 + (c + 1) * 512],
                    start=True,
                    stop=True,
                )
            npop = 1 if r < 12 else (3 if r < 16 else 2)
            pe_jobs = 0
            for _ in range(npop):
                if jobs:
                    j = jobs.popleft()
                    run_job(j)
                    pe_jobs += isinstance(j, tuple) or getattr(j, "pe", False)
            if r >= 12:
                # drain the backlog within the round's PE slack: one pair
                # always, a second on job-free rounds once the queue thins
                want = 1 + (1 if pe_jobs == 0 and r >= 20 else 0)
                n = 0
                while pending and n < want and pending[0]["r"] <= r - 2:
                    emit_pending(pending.popleft())
                    n += 1
            e = epool.tile([128, 1024], F16, tag="e")
            nc.scalar.activation(
                out=e,
                in_=s,
                func=mybir.ActivationFunctionType.Exp,
                scale=1.0 / TEMP,
            )
            pending.append({"r": r, "b": b, "h": h, "kb": kb, "e": e})

        while pending:
            emit_pending(pending.popleft())
        while jobs:
            run_job(jobs.popleft())
    return nc


def build_program():
    nc = bacc.Bacc(None)
    with tile.TileContext(nc) as tc:
        attention_tile_kernel(tc)
    nc.finalize()
    return nc


def kernel(queries: np.ndarray, keys: np.ndarray, values: np.ndarray) -> np.ndarray:
    global _RESULTS
    queries = np.ascontiguousarray(queries, dtype=np.float32)
    keys = np.ascontiguousarray(keys, dtype=np.float32)
    values = np.ascontiguousarray(values, dtype=np.float32)
    nc = build_program()
    in_maps = [
        {
            "q": queries[i * BPC : (i + 1) * BPC],
            "k": keys[i * BPC : (i + 1) * BPC],
            "v": values[i * BPC : (i + 1) * BPC],
        }
        for i in range(NCORES)
    ]
    trace = bool(os.environ.get("ATTN_TRACE"))
    if trace:
        _register_ntff_hook()
    _RESULTS = run_bass_kernel_spmd(nc, in_maps, list(range(NCORES)), trace=trace)
    return np.concatenate([r["out"] for r in _RESULTS.results], axis=0)


def _register_ntff_hook():
    """Dev-only: the slim agent container lacks antenv.axon_hooks; provide it
    so run_bass_kernel_spmd(trace=True) can drive NRT profiling via the axon
    .so directly. No-op unless ATTN_TRACE is set."""
    import sys
    import types

    if "antenv.axon_hooks" in sys.modules:
        return
    try:
        from trn_agent_boot.trn_boot import _ntff_profile_via_ctypes

        h = _ntff_profile_via_ctypes("/opt/axon/libaxon_pjrt.so")
    except Exception:
        return
    mod = types.ModuleType("antenv.axon_hooks")
    mod.get_axon_ntff_profile_hook = lambda: h
    sys.modules["antenv.axon_hooks"] = mod
